# revision 29
# baseline (speedup 1.0000x reference)
"""Trainium2 Bass kernel for the BidderStrategy MLP.

Math (per batch element x, per action n):
    out[n] = b2[n] + sum_h w2[n,h] * relu(w1[n,h] * x + b1[n,h])
    alpha[n] = w3[n, 0]

Since x = uniform[0,1), each hidden unit z = w1*x + b1 is monotone on the
input domain.  Units that never cross zero are either always-off (dropped)
or always-linear (folded, in float64 on host, into a per-action affine
term a_n*x + c_n).  Only the crossing ("active") units are evaluated on
device.  Additionally the unit multiply is folded into the fc2 weight:
    w2 * relu(w1*x + b1) = (w2*w1) * clip(x + b1/w1)
where clip = max(.,0) for w1>0 and min(.,0) for w1<0.  Both are weight-only
transformations - exact (to fp32 noise) for every x in [0,1].

Device kernel (per core, batch-sharded 8 ways, batch tiles of 512):
  - broadcast: one DMA replicates the x tile across all 128 partitions
  - clip: ONE elementwise instruction per j-tile of 128 active units,
    spread across three engines (j-tiles are sign-segregated so each tile
    needs a single clip direction):
      ACT:    activation(Relu, bias=s[p])            (pos tiles only)
      DVE/GP: tensor_scalar(add s[p]; max/min 0)     (2x fp32 SBUF mode)
  - fc2: K=128 fp32 matmuls; the 4 batch subtiles of a supergroup are
    col-tiled into partition bands 32c..32c+12 of ONE psum bank and run
    concurrently on the PE.  The affine fold rides rows 0/1 of j-tile 0
    (x row / ones row, DMA-overwritten after the clip) with lhsT rows
    a_n, c_n + b2_n, so the bias needs no separate pass.  j-tile 0 is
    consumed LAST so the row DMAs sit off the critical path.
  - epilogue: one ACT copy PSUM->SBUF per supergroup, then 4 DMAs out.
"""

import os

import numpy as np

NACT = 12
H = 200
B = 131072
NCORES = 8
B_CORE = B // NCORES  # 16384
BT = 512              # batch tile (matmul free dim, fp32 max)
NBT = B_CORE // BT    # 32
SG = 4                # batch subtiles sharing one output PSUM bank
NSG = NBT // SG       # 8

F32 = np.float32

# Filled by kernel() on each call: BassKernelResults of the last run.
LAST_RESULT = None


def _pack_weights(w1, b1, w2, b2):
    """Classify units and build packed device constants.

    Returns (T, n_pos_tiles, s_pack [128, T], a_pack [128, T*12]).
    j-tiles 0..n_pos_tiles-1 hold w1>0 units (clip = max), the rest hold
    w1<0 units (clip = min).  s_pack[:, t] is the per-partition shift
    b1/w1; a_pack[p, t*12+n] = w2*w1 of the unit at slot p of tile t.
    Rows 0/1 of tile 0 hold the folded linear term a_n and c_n + b2_n.
    """
    w1f = w1[:, :, 0].astype(np.float64)   # [12, 200]
    b1f = b1.astype(np.float64)            # [12, 200]
    w2f = w2[:, 0, :].astype(np.float64)   # [12, 200]
    z0 = b1f
    z1 = w1f + b1f
    zero = np.maximum(z0, z1) <= 0
    linear = (np.minimum(z0, z1) >= 0) & ~zero
    active = ~zero & ~linear

    a_lin = (w2f * w1f * linear).sum(axis=1)                       # [12]
    c_lin = (w2f * b1f * linear).sum(axis=1) + b2[:, 0].astype(np.float64)

    w1_32 = w1[:, :, 0]  # float32 originals: device-exact products
    w2_32 = w2[:, 0, :]
    pos_units = np.argwhere(active & (w1f > 0))   # [(n, h)]
    neg_units = np.argwhere(active & (w1f < 0))
    n_pos, n_neg = len(pos_units), len(neg_units)
    # tile 0 rows 0/1 reserved for the x and ones rows
    n_pos_tiles = 1 + max(0, -(-(n_pos - 126) // 128))
    n_neg_tiles = -(-n_neg // 128)
    T = n_pos_tiles + n_neg_tiles

    s_pack = np.zeros((128, T), dtype=F32)
    a_pack = np.zeros((128, T * 12), dtype=F32)

    slot = 2  # (tile 0, row 2) is the first pos unit slot
    for n, h in pos_units:
        t, p = divmod(slot, 128)
        s_pack[p, t] = F32(b1[n, h]) / F32(w1_32[n, h])
        a_pack[p, t * 12 + n] = F32(w2_32[n, h]) * F32(w1_32[n, h])
        slot += 1
    slot = n_pos_tiles * 128
    for n, h in neg_units:
        t, p = divmod(slot, 128)
        s_pack[p, t] = F32(b1[n, h]) / F32(w1_32[n, h])
        a_pack[p, t * 12 + n] = F32(w2_32[n, h]) * F32(w1_32[n, h])
        slot += 1
    a_pack[0, 0: NACT] = a_lin.astype(F32)
    a_pack[1, 0: NACT] = c_lin.astype(F32)
    return T, n_pos_tiles, s_pack, a_pack


def _build_bass(T, n_pos_tiles, s_pack, a_pack, w3col):
    import concourse.bass as bass
    import concourse.mybir as mybir
    import concourse.tile as tile
    from concourse import bacc

    f32 = mybir.dt.float32
    Relu = mybir.ActivationFunctionType.Relu
    add = mybir.AluOpType.add
    amax = mybir.AluOpType.max
    amin = mybir.AluOpType.min
    nc = bacc.Bacc("TRN2", target_bir_lowering=False, debug=False)

    inp_d = nc.dram_tensor("inp", [B_CORE, 1], f32, kind="ExternalInput")
    out_d = nc.dram_tensor("out", [NACT, B_CORE], f32, kind="ExternalOutput")
    alpha_d = nc.dram_tensor("alpha", [1, NACT], f32, kind="ExternalOutput")

    s_d = nc.inline_tensor(s_pack, name="sc")
    a_d = nc.inline_tensor(a_pack, name="apackc")
    ones_d = nc.inline_tensor(np.ones((1, 2 * BT), dtype=F32), name="onesc")
    w3_d = nc.inline_tensor(w3col.reshape(1, NACT), name="w3c")

    inp_flat = inp_d[:].rearrange("b one -> one b")  # [1, B_CORE]

    # fc2 consumption order: j-tile 0 last, so its post-clip x/ones row
    # DMAs get ~6 matmul rounds of slack.  Clips are produced in a
    # compromise order: the first-consumed tiles first, but j-tile 0
    # early enough that its row DMAs stay off the critical path.
    mm_order = list(range(1, T)) + [0]
    clip_order = list(range(T))

    # engine per (t, pair) clip op, greedy-balanced by measured per-op
    # cost (us) at [128, 1024] width: ACT fused relu ~1.14, DVE
    # tensor_scalar at the 2x fp32 SBUF mode ~0.64.  GPSIMD measured
    # 7.8us/op - unusable.  Neg tiles (min-clip) cannot go on ACT; ACT
    # also runs the epilogue copy (~0.72 per supergroup).
    NPAIR = SG // 2
    cost = {"a": 1.14, "v": 0.70}
    load = {"a": 0.72, "v": 0.0}  # ACT pre-loaded with epilogue
    schedule = {}
    for t in range(T):
        for c2 in range(NPAIR):
            allowed = ["a", "v"] if t < n_pos_tiles else ["v"]
            eng = min(allowed, key=lambda e: load[e] + cost[e])
            load[eng] += cost[eng]
            schedule[(t, c2)] = eng

    with tile.TileContext(nc) as tc:
        with (
            tc.tile_pool(name="consts", bufs=1) as consts,
            tc.tile_pool(name="xbp", bufs=4) as xbp,
            tc.tile_pool(name="gp", bufs=12) as gp,
            tc.tile_pool(name="outp", bufs=2) as outp,
            tc.tile_pool(name="pop", bufs=2, space="PSUM") as pop,
        ):
            s_sb = consts.tile([128, T], f32)
            nc.sync.dma_start(out=s_sb[:], in_=s_d[:])
            a_sb = consts.tile([128, T * 12], f32)
            nc.sync.dma_start(out=a_sb[:], in_=a_d[:])

            al_sb = consts.tile([1, NACT], f32)
            nc.sync.dma_start(out=al_sb[:], in_=w3_d[:])
            nc.sync.dma_start(out=alpha_d[:], in_=al_sb[:])

            PW = BT * 2  # pair width: one clip op covers 2 batch subtiles
            for sg in range(NSG):
                po = pop.tile([128, BT], f32)
                # sg 0 uses narrow per-subtile chunks so the first
                # broadcast DMA (gating kernel startup) is 4x smaller and
                # the first matmul starts ~6us earlier; later supergroups
                # use pair-width chunks for lower per-op overhead.
                W = BT if sg == 0 else PW
                NCH = (SG * BT) // W
                xbs = []
                gs = []
                for ch in range(NCH):
                    off = sg * SG * BT + ch * W
                    xb = xbp.tile([128, W], f32, name="xb")
                    # replicate the x chunk into all 128 partitions
                    nc.sync.dma_start(
                        out=xb[:],
                        in_=bass.AP(inp_d, off, [[0, 128], [1, W]]),
                    )
                    xbs.append(xb)
                    gs.append([None] * T)
                # single-instruction clip per (t, chunk), ACT/DVE balanced
                for t in clip_order:
                    cl = amax if t < n_pos_tiles else amin
                    for ch in range(NCH):
                        g = gp.tile([128, W], f32, name="g")
                        gs[ch][t] = g
                        s_ap = s_sb[:, t: t + 1]
                        if schedule[(t, (ch * W) // PW)] == "a":
                            nc.scalar.activation(
                                g[:], xbs[ch][:], Relu, bias=s_ap, scale=1.0
                            )
                        else:
                            nc.vector.tensor_scalar(
                                g[:], xbs[ch][:], s_ap, 0.0, add, cl
                            )
                        if t == 0:
                            off = sg * SG * BT + ch * W
                            nc.sync.dma_start(
                                out=g[0:1, :],
                                in_=inp_flat[:, off: off + W],
                            )
                            nc.sync.dma_start(
                                out=g[1:2, :], in_=ones_d[:, 0: W]
                            )
                # fc2: col-tiled accumulation, 4 subtiles concurrent
                for ti, t in enumerate(mm_order):
                    for c in range(SG):
                        ch, rem = divmod(c * BT, W)
                        nc.tensor.matmul(
                            po[32 * c: 32 * c + NACT, :],
                            a_sb[:, t * 12: (t + 1) * 12],
                            gs[ch][t][:, rem: rem + BT],
                            start=(ti == 0),
                            stop=(ti == T - 1),
                            tile_position=(0, 32 * c),
                        )
                osb = outp.tile([128, BT], f32)
                nc.scalar.copy(osb[:], po[:])
                for c in range(SG):
                    bt = sg * SG + c
                    nc.sync.dma_start(
                        out=out_d[:, bt * BT: (bt + 1) * BT],
                        in_=osb[32 * c: 32 * c + NACT, :],
                    )

    nc.compile()
    return nc


def kernel(inp, w1, b1, w2, b2, w3):
    global LAST_RESULT
    from concourse.bass_utils import run_bass_kernel_spmd

    inp = np.ascontiguousarray(np.asarray(inp, dtype=F32))
    w1 = np.asarray(w1, dtype=F32)
    b1 = np.asarray(b1, dtype=F32)
    w2 = np.asarray(w2, dtype=F32)
    b2 = np.asarray(b2, dtype=F32)
    w3 = np.asarray(w3, dtype=F32)

    T, n_pos_tiles, s_pack, a_pack = _pack_weights(w1, b1, w2, b2)
    nc = _build_bass(T, n_pos_tiles, s_pack, a_pack, w3[:, 0].copy())

    in_maps = [
        {"inp": inp[i * B_CORE: (i + 1) * B_CORE]} for i in range(NCORES)
    ]
    trace = bool(int(os.environ.get("KERNEL_TRACE", "0")))
    res = run_bass_kernel_spmd(
        nc, in_maps, core_ids=list(range(NCORES)), trace=trace
    )
    LAST_RESULT = res

    out = np.concatenate([res.results[i]["out"] for i in range(NCORES)], axis=1)
    alpha = res.results[0]["alpha"].reshape(NACT).astype(F32)
    return alpha, out


# revision 33
# speedup vs baseline: 1.0116x; 1.0116x over previous
"""Trainium2 Bass kernel for the BidderStrategy MLP.

Math (per batch element x, per action n):
    out[n] = b2[n] + sum_h w2[n,h] * relu(w1[n,h] * x + b1[n,h])
    alpha[n] = w3[n, 0]

Since x = uniform[0,1), each hidden unit z = w1*x + b1 is monotone on the
input domain.  Units that never cross zero are either always-off (dropped)
or always-linear (folded, in float64 on host, into a per-action affine
term a_n*x + c_n).  Only the crossing ("active") units are evaluated on
device.  Additionally the unit multiply is folded into the fc2 weight:
    w2 * relu(w1*x + b1) = (w2*w1) * clip(x + b1/w1)
where clip = max(.,0) for w1>0 and min(.,0) for w1<0.  Both are weight-only
transformations - exact (to fp32 noise) for every x in [0,1].

Device kernel (per core, batch-sharded 8 ways, batch tiles of 512):
  - broadcast: one DMA replicates the x tile across all 128 partitions
  - clip: ONE elementwise instruction per j-tile of 128 active units,
    spread across three engines (j-tiles are sign-segregated so each tile
    needs a single clip direction):
      ACT:    activation(Relu, bias=s[p])            (pos tiles only)
      DVE/GP: tensor_scalar(add s[p]; max/min 0)     (2x fp32 SBUF mode)
  - fc2: K=128 fp32 matmuls; the 4 batch subtiles of a supergroup are
    col-tiled into partition bands 32c..32c+12 of ONE psum bank and run
    concurrently on the PE.  The affine fold rides rows 0/1 of j-tile 0
    (x row / ones row, DMA-overwritten after the clip) with lhsT rows
    a_n, c_n + b2_n, so the bias needs no separate pass.  j-tile 0 is
    consumed LAST so the row DMAs sit off the critical path.
  - epilogue: one ACT copy PSUM->SBUF per supergroup, then 4 DMAs out.
"""

import os

import numpy as np

NACT = 12
H = 200
B = 131072
NCORES = 8
B_CORE = B // NCORES  # 16384
BT = 512              # batch tile (matmul free dim, fp32 max)
NBT = B_CORE // BT    # 32
SG = 4                # batch subtiles sharing one output PSUM bank
NSG = NBT // SG       # 8

F32 = np.float32

# Filled by kernel() on each call: BassKernelResults of the last run.
LAST_RESULT = None


def _pack_weights(w1, b1, w2, b2):
    """Classify units and build packed device constants.

    Returns (T, n_pos_tiles, s_pack [128, T], a_pack [128, T*12]).
    j-tiles 0..n_pos_tiles-1 hold w1>0 units (clip = max), the rest hold
    w1<0 units (clip = min).  s_pack[:, t] is the per-partition shift
    b1/w1; a_pack[p, t*12+n] = w2*w1 of the unit at slot p of tile t.
    Rows 0/1 of tile 0 hold the folded linear term a_n and c_n + b2_n.
    """
    w1f = w1[:, :, 0].astype(np.float64)   # [12, 200]
    b1f = b1.astype(np.float64)            # [12, 200]
    w2f = w2[:, 0, :].astype(np.float64)   # [12, 200]
    z0 = b1f
    z1 = w1f + b1f
    zero = np.maximum(z0, z1) <= 0
    linear = (np.minimum(z0, z1) >= 0) & ~zero
    active = ~zero & ~linear

    a_lin = (w2f * w1f * linear).sum(axis=1)                       # [12]
    c_lin = (w2f * b1f * linear).sum(axis=1) + b2[:, 0].astype(np.float64)

    w1_32 = w1[:, :, 0]  # float32 originals: device-exact products
    w2_32 = w2[:, 0, :]
    pos_units = np.argwhere(active & (w1f > 0))   # [(n, h)]
    neg_units = np.argwhere(active & (w1f < 0))
    n_pos, n_neg = len(pos_units), len(neg_units)
    # tile 0 rows 0/1 reserved for the x and ones rows
    n_pos_tiles = 1 + max(0, -(-(n_pos - 126) // 128))
    n_neg_tiles = -(-n_neg // 128)
    T = n_pos_tiles + n_neg_tiles

    s_pack = np.zeros((128, T), dtype=F32)
    a_pack = np.zeros((128, T * 12), dtype=F32)

    slot = 2  # (tile 0, row 2) is the first pos unit slot
    for n, h in pos_units:
        t, p = divmod(slot, 128)
        s_pack[p, t] = F32(b1[n, h]) / F32(w1_32[n, h])
        a_pack[p, t * 12 + n] = F32(w2_32[n, h]) * F32(w1_32[n, h])
        slot += 1
    slot = n_pos_tiles * 128
    for n, h in neg_units:
        t, p = divmod(slot, 128)
        s_pack[p, t] = F32(b1[n, h]) / F32(w1_32[n, h])
        a_pack[p, t * 12 + n] = F32(w2_32[n, h]) * F32(w1_32[n, h])
        slot += 1
    a_pack[0, 0: NACT] = a_lin.astype(F32)
    a_pack[1, 0: NACT] = c_lin.astype(F32)
    return T, n_pos_tiles, s_pack, a_pack


def _build_bass(T, n_pos_tiles, s_pack, a_pack, w3col):
    import concourse.bass as bass
    import concourse.mybir as mybir
    import concourse.tile as tile
    from concourse import bacc

    f32 = mybir.dt.float32
    Relu = mybir.ActivationFunctionType.Relu
    add = mybir.AluOpType.add
    amax = mybir.AluOpType.max
    amin = mybir.AluOpType.min
    nc = bacc.Bacc("TRN2", target_bir_lowering=False, debug=False)

    inp_d = nc.dram_tensor("inp", [B_CORE, 1], f32, kind="ExternalInput")
    out_d = nc.dram_tensor("out", [NACT, B_CORE], f32, kind="ExternalOutput")
    alpha_d = nc.dram_tensor("alpha", [1, NACT], f32, kind="ExternalOutput")

    s_d = nc.inline_tensor(s_pack, name="sc")
    a_d = nc.inline_tensor(a_pack, name="apackc")
    ones_d = nc.inline_tensor(np.ones((1, 2 * BT), dtype=F32), name="onesc")
    w3_d = nc.inline_tensor(w3col.reshape(1, NACT), name="w3c")

    inp_flat = inp_d[:].rearrange("b one -> one b")  # [1, B_CORE]

    # fc2 consumption order: j-tile 0 last, so its post-clip x/ones row
    # DMAs get ~6 matmul rounds of slack.  Clips are produced in a
    # compromise order: the first-consumed tiles first, but j-tile 0
    # early enough that its row DMAs stay off the critical path.
    mm_order = list(range(1, T)) + [0]
    clip_order = list(range(T))

    # engine per (t, pair) clip op, greedy-balanced by measured per-op
    # cost (us) at [128, 1024] width: ACT fused relu ~1.14, DVE
    # tensor_scalar at the 2x fp32 SBUF mode ~0.64.  GPSIMD measured
    # 7.8us/op - unusable.  Neg tiles (min-clip) cannot go on ACT; ACT
    # also runs the epilogue copy (~0.72 per supergroup).
    NPAIR = SG // 2
    cost = {"a": 1.14, "v": 0.64}
    load = {"a": 0.72, "v": 0.0}  # ACT pre-loaded with epilogue
    schedule = {}
    for t in range(T):
        for c2 in range(NPAIR):
            allowed = ["a", "v"] if t < n_pos_tiles else ["v"]
            eng = min(allowed, key=lambda e: load[e] + cost[e])
            load[eng] += cost[eng]
            schedule[(t, c2)] = eng

    with tile.TileContext(nc) as tc:
        with (
            tc.tile_pool(name="consts", bufs=1) as consts,
            tc.tile_pool(name="xbp", bufs=4) as xbp,
            tc.tile_pool(name="gp", bufs=12) as gp,
            tc.tile_pool(name="outp", bufs=2) as outp,
            tc.tile_pool(name="pop", bufs=2, space="PSUM") as pop,
            tc.tile_pool(name="bcp", bufs=2, space="PSUM") as bcp,
        ):
            s_sb = consts.tile([128, T], f32)
            nc.sync.dma_start(out=s_sb[:], in_=s_d[:])
            a_sb = consts.tile([128, T * 12], f32)
            nc.sync.dma_start(out=a_sb[:], in_=a_d[:])

            al_sb = consts.tile([1, NACT], f32)
            nc.sync.dma_start(out=al_sb[:], in_=w3_d[:])
            nc.sync.dma_start(out=alpha_d[:], in_=al_sb[:])

            # ones row in SBUF: lhsT for the supergroup-0 PE broadcast
            ones_sb = consts.tile([1, 2 * BT], f32)
            nc.sync.dma_start(out=ones_sb[:], in_=ones_d[:])

            PW = BT * 2  # pair width: one clip op covers 2 batch subtiles
            for sg in range(NSG):
                po = pop.tile([128, BT], f32)
                xbs = []
                gs = []
                for c2 in range(NPAIR):
                    off = (sg * SG + 2 * c2) * BT
                    xb = xbp.tile([128, PW], f32, name="xb")
                    if sg == 0:
                        # The DMA replicate is descriptor-bound (~6.5us for
                        # 128 partition descriptors) and gates kernel start.
                        # For the first supergroup broadcast via the (idle)
                        # PE instead: a 4KB x-row DMA, then ones (x) x into
                        # PSUM, then ACT/DVE copies to SBUF.
                        xrow = xbp.tile([1, PW], f32, name="xrow")
                        nc.sync.dma_start(
                            out=xrow[:], in_=inp_flat[:, off: off + PW]
                        )
                        for half in range(2):
                            pb = bcp.tile([128, BT], f32)
                            nc.tensor.matmul(
                                pb[:],
                                ones_sb[0:1, 0:128],
                                xrow[:, half * BT: (half + 1) * BT],
                                start=True,
                                stop=True,
                            )
                            dst = xb[:, half * BT: (half + 1) * BT]
                            if half == 0:
                                nc.scalar.copy(dst, pb[:])
                            else:
                                nc.vector.tensor_copy(out=dst, in_=pb[:])
                    else:
                        # replicate the x pair into all 128 partitions
                        nc.sync.dma_start(
                            out=xb[:],
                            in_=bass.AP(inp_d, off, [[0, 128], [1, PW]]),
                        )
                    xbs.append(xb)
                    gs.append([None] * T)
                # single-instruction clip per (t, pair), ACT/DVE balanced
                for t in clip_order:
                    cl = amax if t < n_pos_tiles else amin
                    for c2 in range(NPAIR):
                        g = gp.tile([128, PW], f32)
                        gs[c2][t] = g
                        s_ap = s_sb[:, t: t + 1]
                        if schedule[(t, c2)] == "a":
                            nc.scalar.activation(
                                g[:], xbs[c2][:], Relu, bias=s_ap, scale=1.0
                            )
                        else:
                            nc.vector.tensor_scalar(
                                g[:], xbs[c2][:], s_ap, 0.0, add, cl
                            )
                        if t == 0:
                            off = (sg * SG + 2 * c2) * BT
                            nc.sync.dma_start(
                                out=g[0:1, :],
                                in_=inp_flat[:, off: off + PW],
                            )
                            nc.sync.dma_start(out=g[1:2, :], in_=ones_d[:])
                # fc2: col-tiled accumulation, 4 subtiles concurrent
                for ti, t in enumerate(mm_order):
                    for c in range(SG):
                        c2, half = divmod(c, 2)
                        nc.tensor.matmul(
                            po[32 * c: 32 * c + NACT, :],
                            a_sb[:, t * 12: (t + 1) * 12],
                            gs[c2][t][:, half * BT: (half + 1) * BT],
                            start=(ti == 0),
                            stop=(ti == T - 1),
                            tile_position=(0, 32 * c),
                        )
                osb = outp.tile([128, BT], f32)
                nc.scalar.copy(osb[:], po[:])
                for c in range(SG):
                    bt = sg * SG + c
                    nc.sync.dma_start(
                        out=out_d[:, bt * BT: (bt + 1) * BT],
                        in_=osb[32 * c: 32 * c + NACT, :],
                    )

    nc.compile()
    return nc


def kernel(inp, w1, b1, w2, b2, w3):
    global LAST_RESULT
    from concourse.bass_utils import run_bass_kernel_spmd

    inp = np.ascontiguousarray(np.asarray(inp, dtype=F32))
    w1 = np.asarray(w1, dtype=F32)
    b1 = np.asarray(b1, dtype=F32)
    w2 = np.asarray(w2, dtype=F32)
    b2 = np.asarray(b2, dtype=F32)
    w3 = np.asarray(w3, dtype=F32)

    T, n_pos_tiles, s_pack, a_pack = _pack_weights(w1, b1, w2, b2)
    nc = _build_bass(T, n_pos_tiles, s_pack, a_pack, w3[:, 0].copy())

    in_maps = [
        {"inp": inp[i * B_CORE: (i + 1) * B_CORE]} for i in range(NCORES)
    ]
    trace = bool(int(os.environ.get("KERNEL_TRACE", "0")))
    res = run_bass_kernel_spmd(
        nc, in_maps, core_ids=list(range(NCORES)), trace=trace
    )
    LAST_RESULT = res

    out = np.concatenate([res.results[i]["out"] for i in range(NCORES)], axis=1)
    alpha = res.results[0]["alpha"].reshape(NACT).astype(F32)
    return alpha, out


# revision 38
# speedup vs baseline: 1.2214x; 1.2074x over previous
"""Trainium2 Bass kernel for the BidderStrategy MLP.

Math (per batch element x, per action n):
    out[n] = b2[n] + sum_h w2[n,h] * relu(w1[n,h] * x + b1[n,h])
    alpha[n] = w3[n, 0]

Since x = uniform[0,1), each hidden unit z = w1*x + b1 is monotone on the
input domain.  Units that never cross zero are either always-off (dropped)
or always-linear (folded, in float64 on host, into a per-action affine
term a_n*x + c_n).  Only the crossing ("active") units are evaluated on
device.  Additionally the unit multiply is folded into the fc2 weight:
    w2 * relu(w1*x + b1) = (w2*w1) * clip(x + b1/w1)
where clip = max(.,0) for w1>0 and min(.,0) for w1<0.  Both are weight-only
transformations - exact (to fp32 noise) for every x in [0,1].

Device kernel (per core, batch-sharded 8 ways, batch tiles of 512):
  - broadcast: one DMA replicates the x tile across all 128 partitions
  - clip: ONE elementwise instruction per j-tile of 128 active units,
    spread across three engines (j-tiles are sign-segregated so each tile
    needs a single clip direction):
      ACT:    activation(Relu, bias=s[p])            (pos tiles only)
      DVE/GP: tensor_scalar(add s[p]; max/min 0)     (2x fp32 SBUF mode)
  - fc2: K=128 fp32 matmuls; the 4 batch subtiles of a supergroup are
    col-tiled into partition bands 32c..32c+12 of ONE psum bank and run
    concurrently on the PE.  The affine fold rides rows 0/1 of j-tile 0
    (x row / ones row, DMA-overwritten after the clip) with lhsT rows
    a_n, c_n + b2_n, so the bias needs no separate pass.  j-tile 0 is
    consumed LAST so the row DMAs sit off the critical path.
  - epilogue: one ACT copy PSUM->SBUF per supergroup, then 4 DMAs out.
"""

import os

import numpy as np

NACT = 12
H = 200
B = 131072
NCORES = 8
B_CORE = B // NCORES  # 16384
BT = 512              # batch tile (matmul free dim, fp32 max)
NBT = B_CORE // BT    # 32
SG = 4                # batch subtiles sharing one output PSUM bank
NSG = NBT // SG       # 8

F32 = np.float32

# Filled by kernel() on each call: BassKernelResults of the last run.
LAST_RESULT = None


def _pack_weights(w1, b1, w2, b2):
    """Classify units and build packed device constants.

    Returns (T, n_pos_tiles, s_pack [128, T], a_pack [128, T*12]).
    j-tiles 0..n_pos_tiles-1 hold w1>0 units (clip = max), the rest hold
    w1<0 units (clip = min).  s_pack[:, t] is the per-partition shift
    b1/w1; a_pack[p, t*12+n] = w2*w1 of the unit at slot p of tile t.
    Rows 0/1 of tile 0 hold the folded linear term a_n and c_n + b2_n.
    """
    w1f = w1[:, :, 0].astype(np.float64)   # [12, 200]
    b1f = b1.astype(np.float64)            # [12, 200]
    w2f = w2[:, 0, :].astype(np.float64)   # [12, 200]
    z0 = b1f
    z1 = w1f + b1f
    zero = np.maximum(z0, z1) <= 0
    linear = (np.minimum(z0, z1) >= 0) & ~zero
    active = ~zero & ~linear

    a_lin = (w2f * w1f * linear).sum(axis=1)                       # [12]
    c_lin = (w2f * b1f * linear).sum(axis=1) + b2[:, 0].astype(np.float64)

    w1_32 = w1[:, :, 0]  # float32 originals: device-exact products
    w2_32 = w2[:, 0, :]
    pos_units = np.argwhere(active & (w1f > 0))   # [(n, h)]
    neg_units = np.argwhere(active & (w1f < 0))
    n_pos, n_neg = len(pos_units), len(neg_units)
    # tile 0 rows 0/1 reserved for the x and ones rows
    n_pos_tiles = 1 + max(0, -(-(n_pos - 126) // 128))
    n_neg_tiles = -(-n_neg // 128)
    T = n_pos_tiles + n_neg_tiles

    s_pack = np.zeros((128, T), dtype=F32)
    a_pack = np.zeros((128, T * 12), dtype=F32)

    slot = 2  # (tile 0, row 2) is the first pos unit slot
    for n, h in pos_units:
        t, p = divmod(slot, 128)
        s_pack[p, t] = F32(b1[n, h]) / F32(w1_32[n, h])
        a_pack[p, t * 12 + n] = F32(w2_32[n, h]) * F32(w1_32[n, h])
        slot += 1
    slot = n_pos_tiles * 128
    for n, h in neg_units:
        t, p = divmod(slot, 128)
        s_pack[p, t] = F32(b1[n, h]) / F32(w1_32[n, h])
        a_pack[p, t * 12 + n] = F32(w2_32[n, h]) * F32(w1_32[n, h])
        slot += 1
    a_pack[0, 0: NACT] = a_lin.astype(F32)
    a_pack[1, 0: NACT] = c_lin.astype(F32)
    return T, n_pos_tiles, s_pack, a_pack


def _build_bass(T, n_pos_tiles, s_pack, a_pack, w3col):
    import concourse.bass as bass
    import concourse.mybir as mybir
    import concourse.tile as tile
    from concourse import bacc

    f32 = mybir.dt.float32
    Relu = mybir.ActivationFunctionType.Relu
    add = mybir.AluOpType.add
    amax = mybir.AluOpType.max
    amin = mybir.AluOpType.min
    nc = bacc.Bacc("TRN2", target_bir_lowering=False, debug=False)

    inp_d = nc.dram_tensor("inp", [B_CORE, 1], f32, kind="ExternalInput")
    out_d = nc.dram_tensor("out", [NACT, B_CORE], f32, kind="ExternalOutput")
    alpha_d = nc.dram_tensor("alpha", [1, NACT], f32, kind="ExternalOutput")

    s_d = nc.inline_tensor(s_pack, name="sc")
    a_d = nc.inline_tensor(a_pack, name="apackc")
    ones_d = nc.inline_tensor(np.ones((1, SG * BT), dtype=F32), name="onesc")
    w3_d = nc.inline_tensor(w3col.reshape(1, NACT), name="w3c")

    inp_flat = inp_d[:].rearrange("b one -> one b")  # [1, B_CORE]

    # fc2 consumption order: j-tile 0 last, so its post-clip x/ones row
    # DMAs get ~6 matmul rounds of slack.  Clips are produced in a
    # compromise order: the first-consumed tiles first, but j-tile 0
    # early enough that its row DMAs stay off the critical path.
    mm_order = list(range(1, T)) + [0]
    clip_order = list(range(T))

    # engine per (t, pair) clip op, greedy-balanced by measured per-op
    # cost (us) at [128, 1024] width: ACT fused relu ~1.14, DVE
    # tensor_scalar at the 2x fp32 SBUF mode ~0.64.  GPSIMD measured
    # 7.8us/op - unusable.  Neg tiles (min-clip) cannot go on ACT; ACT
    # also runs the epilogue copy (~0.72 per supergroup).
    # one full-supergroup-width clip op per j-tile: amortizes per-op
    # overhead and makes each matmul round single-producer (all 4
    # concurrent matmuls of a round become ready at once)
    NCH = 1
    cost = {"a": 1.98, "v": 1.28}
    load = {"a": 0.72, "v": 0.0}  # ACT pre-loaded with epilogue
    schedule = {}
    for t in range(T):
        for c2 in range(NCH):
            allowed = ["a", "v"] if t < n_pos_tiles else ["v"]
            eng = min(allowed, key=lambda e: load[e] + cost[e])
            load[eng] += cost[eng]
            schedule[(t, c2)] = eng

    with tile.TileContext(nc) as tc:
        with (
            tc.tile_pool(name="consts", bufs=1) as consts,
            tc.tile_pool(name="xbp", bufs=4) as xbp,
            tc.tile_pool(name="gp", bufs=12) as gp,
            tc.tile_pool(name="outp", bufs=2) as outp,
            tc.tile_pool(name="pop", bufs=2, space="PSUM") as pop,
        ):
            s_sb = consts.tile([128, T], f32)
            nc.sync.dma_start(out=s_sb[:], in_=s_d[:])
            a_sb = consts.tile([128, T * 12], f32)
            nc.sync.dma_start(out=a_sb[:], in_=a_d[:])

            al_sb = consts.tile([1, NACT], f32)
            nc.sync.dma_start(out=al_sb[:], in_=w3_d[:])
            nc.sync.dma_start(out=alpha_d[:], in_=al_sb[:])

            PW = BT * SG  # one clip op covers the whole supergroup
            for sg in range(NSG):
                po = pop.tile([128, BT], f32)
                xbs = []
                gs = []
                for c2 in range(NCH):
                    off = (sg * SG) * BT + c2 * PW
                    xb = xbp.tile([128, PW], f32)
                    # replicate the supergroup's x into all partitions
                    # (cost is descriptor-bound: width is free)
                    nc.sync.dma_start(
                        out=xb[:],
                        in_=bass.AP(inp_d, off, [[0, 128], [1, PW]]),
                    )
                    xbs.append(xb)
                    gs.append([None] * T)
                # single-instruction clip per (t, chunk), ACT/DVE balanced
                for t in clip_order:
                    cl = amax if t < n_pos_tiles else amin
                    for c2 in range(NCH):
                        g = gp.tile([128, PW], f32)
                        gs[c2][t] = g
                        s_ap = s_sb[:, t: t + 1]
                        if schedule[(t, c2)] == "a":
                            nc.scalar.activation(
                                g[:], xbs[c2][:], Relu, bias=s_ap, scale=1.0
                            )
                        else:
                            nc.vector.tensor_scalar(
                                g[:], xbs[c2][:], s_ap, 0.0, add, cl
                            )
                        if t == 0:
                            off = (sg * SG) * BT + c2 * PW
                            nc.sync.dma_start(
                                out=g[0:1, :],
                                in_=inp_flat[:, off: off + PW],
                            )
                            nc.sync.dma_start(out=g[1:2, :], in_=ones_d[:])
                # fc2: col-tiled accumulation, 4 subtiles concurrent
                for ti, t in enumerate(mm_order):
                    for c in range(SG):
                        ch, rem = divmod(c * BT, PW)
                        nc.tensor.matmul(
                            po[32 * c: 32 * c + NACT, :],
                            a_sb[:, t * 12: (t + 1) * 12],
                            gs[ch][t][:, rem: rem + BT],
                            start=(ti == 0),
                            stop=(ti == T - 1),
                            tile_position=(0, 32 * c),
                        )
                osb = outp.tile([128, BT], f32)
                nc.scalar.copy(osb[:], po[:])
                for c in range(SG):
                    bt = sg * SG + c
                    nc.sync.dma_start(
                        out=out_d[:, bt * BT: (bt + 1) * BT],
                        in_=osb[32 * c: 32 * c + NACT, :],
                    )

    nc.compile()
    return nc


def kernel(inp, w1, b1, w2, b2, w3):
    global LAST_RESULT
    from concourse.bass_utils import run_bass_kernel_spmd

    inp = np.ascontiguousarray(np.asarray(inp, dtype=F32))
    w1 = np.asarray(w1, dtype=F32)
    b1 = np.asarray(b1, dtype=F32)
    w2 = np.asarray(w2, dtype=F32)
    b2 = np.asarray(b2, dtype=F32)
    w3 = np.asarray(w3, dtype=F32)

    T, n_pos_tiles, s_pack, a_pack = _pack_weights(w1, b1, w2, b2)
    nc = _build_bass(T, n_pos_tiles, s_pack, a_pack, w3[:, 0].copy())

    in_maps = [
        {"inp": inp[i * B_CORE: (i + 1) * B_CORE]} for i in range(NCORES)
    ]
    trace = bool(int(os.environ.get("KERNEL_TRACE", "0")))
    res = run_bass_kernel_spmd(
        nc, in_maps, core_ids=list(range(NCORES)), trace=trace
    )
    LAST_RESULT = res

    out = np.concatenate([res.results[i]["out"] for i in range(NCORES)], axis=1)
    alpha = res.results[0]["alpha"].reshape(NACT).astype(F32)
    return alpha, out
